# revision 15
# baseline (speedup 1.0000x reference)
"""nn_PointCloud2Mesh kernel for 8 trn2 NeuronCores.

Pipeline: host voxelize (O(N) binning) -> device pass 1 (conv1 -> BN with
cross-core stats allreduce -> ReLU -> conv2 = offset field) -> host trilinear
grid_sample -> device pass 2 (same NEFF: decoder convs) -> host sigmoid.

Sharding: core c of 8 handles batch c//2, z-slab c%2 (32 slices + halo).
Both device passes run one shared Bass NEFF on cores 0-7 via PJRT; BN uses
an 8-core AllReduce of per-channel sums.  Heavy compute (the 22 GFLOP of
3^3 convs) runs on the NeuronCores; scatter/gather stay on host where they
are O(N) cheap.

A numpy fallback covers any device-path failure.
"""
import os
import numpy as np

G = 64
B, N = 4, 200000
YX = G * G
PYX = 66 * 66
N_CORES = 8
EPS = 1e-5

# ---------------------------------------------------------------------------
# host-side reference pieces (voxelize / grid_sample) - cheap O(N) parts
# ---------------------------------------------------------------------------


def _np_voxelize(points):
    pmin = points.min(axis=1, keepdims=True)
    pmax = points.max(axis=1, keepdims=True)
    npts = (points - pmin) / (pmax - pmin + 1e-6) * 2.0 - 1.0
    # npts >= -1 so (npts+1)*0.5*G >= 0: int cast == floor
    idx = np.clip(((npts + 1.0) * (0.5 * G)).astype(np.int32), 0, G - 1)
    lin = (idx[..., 0] * G + idx[..., 1]) * G + idx[..., 2]
    nb = points.shape[0]
    lin = lin + (np.arange(nb, dtype=np.int64)[:, None] * (G * G * G))
    hist = np.bincount(lin.ravel(), minlength=nb * G * G * G)
    return hist.astype(np.float32).reshape(nb, G, G, G)


def _np_grid_sample(vol, grid):
    Bv, C, D, H, W = vol.shape

    def unnorm(c, size):
        u = ((c + 1.0) * size - 1.0) * 0.5
        return np.clip(u, 0.0, size - 1.0)

    ix = unnorm(grid[..., 0], W)
    iy = unnorm(grid[..., 1], H)
    iz = unnorm(grid[..., 2], D)
    ix0, iy0, iz0 = np.floor(ix), np.floor(iy), np.floor(iz)
    fx, fy, fz = ix - ix0, iy - iy0, iz - iz0
    flat = vol.reshape(Bv, C, -1)
    zc = [np.clip(iz0.astype(np.int32), 0, D - 1) * (H * W),
          np.clip(iz0.astype(np.int32) + 1, 0, D - 1) * (H * W)]
    yc = [np.clip(iy0.astype(np.int32), 0, H - 1) * W,
          np.clip(iy0.astype(np.int32) + 1, 0, H - 1) * W]
    xc = [np.clip(ix0.astype(np.int32), 0, W - 1),
          np.clip(ix0.astype(np.int32) + 1, 0, W - 1)]
    wzs = [1.0 - fz, fz]
    wys = [1.0 - fy, fy]
    wxs = [1.0 - fx, fx]
    out = np.zeros_like(vol)
    for kz in range(2):
        for ky in range(2):
            zy = zc[kz] + yc[ky]
            wzy = wzs[kz] * wys[ky]
            for kx in range(2):
                lin = (zy + xc[kx]).reshape(Bv, -1)
                g = np.take_along_axis(flat, lin[:, None, :], axis=2)
                out += g.reshape(vol.shape) * (wzy * wxs[kx])[:, None]
    return out


# ---------------------------------------------------------------------------
# Bass kernel (built lazily; shared by encoder and decoder passes)
# ---------------------------------------------------------------------------


def _build_nc():
    import concourse.bass as bass
    import concourse.mybir as mybir
    from concourse.tile import TileContext

    F32 = mybir.dt.float32
    AF = mybir.ActivationFunctionType
    OP = mybir.AluOpType
    NVOX_STATS = float(4 * G * G * G)

    nc = bass.Bass("TRN2", target_bir_lowering=False,
                   disable_frame_to_traceback=True)

    # vol row r (r=0..35) = padded z index (z0-1+r) of the 66^3 zero-padded
    # volume (rows outside [0,66) zero).  h slice j (0..33) = conv1 output at
    # global z = z0-1+j, from vol rows j..j+2.
    F16i = mybir.dt.float16
    vol = nc.dram_tensor("vol", [36, PYX], F16i, kind="ExternalInput")
    w1 = nc.dram_tensor("w1", [27, 64], F16i, kind="ExternalInput")
    b1 = nc.dram_tensor("b1", [64, 1], F32, kind="ExternalInput")
    gamma = nc.dram_tensor("gamma", [64, 1], F32, kind="ExternalInput")
    beta = nc.dram_tensor("beta", [64, 1], F32, kind="ExternalInput")
    w2 = nc.dram_tensor("w2", [64, 81], F32, kind="ExternalInput")
    b2 = nc.dram_tensor("b2", [3, 1], F32, kind="ExternalInput")
    hmask = nc.dram_tensor("hmask", [64, 34], F32, kind="ExternalInput")
    BF16 = mybir.dt.bfloat16
    F16 = mybir.dt.float16
    out = nc.dram_tensor("out", [3, 16 * YX], F16, kind="ExternalOutput")
    outh = nc.dram_tensor("outh", [3, 16 * YX], F16, kind="ExternalOutput")
    # channel-0 occupancy = sigmoid(logit) written separately so the decoder
    # pass only downloads 1/3 of the bytes
    occ = nc.dram_tensor("occ", [1, 32 * YX], F16, kind="ExternalOutput")

    h_raw = nc.dram_tensor("h_raw", [34, 64, YX], F32)
    st_in = nc.dram_tensor("st_in", [64, 2], F32)
    st_out = nc.dram_tensor("st_out", [64, 2], F32)

    with TileContext(nc) as tc:
        with (
            tc.tile_pool(name="im2col", bufs=2) as p_im,
            tc.tile_pool(name="psum", bufs=4, space="PSUM") as p_ps,
            tc.tile_pool(name="hout", bufs=2) as p_h,
            tc.tile_pool(name="consts", bufs=1) as p_c,
            tc.tile_pool(name="stats", bufs=1) as p_st,
            tc.tile_pool(name="ring", bufs=1) as p_ring,
            tc.tile_pool(name="o2", bufs=2) as p_o2,
        ):
            w1_t = p_c.tile([27, 64], F16i)
            nc.sync.dma_start(out=w1_t[:], in_=w1[:, :])
            w2_t = p_c.tile([64, 81], F32)
            nc.sync.dma_start(out=w2_t[:], in_=w2[:, :])
            b1_t = p_c.tile([64, 1], F32)
            nc.sync.dma_start(out=b1_t[:], in_=b1[:, :])
            gamma_t = p_c.tile([64, 1], F32)
            nc.sync.dma_start(out=gamma_t[:], in_=gamma[:, :])
            beta_t = p_c.tile([64, 1], F32)
            nc.sync.dma_start(out=beta_t[:], in_=beta[:, :])
            b2_t = p_c.tile([3, 1], F32)
            nc.sync.dma_start(out=b2_t[:], in_=b2[:, :])
            hm_t = p_c.tile([64, 34], F32)
            nc.sync.dma_start(out=hm_t[:], in_=hmask[:, :])

            ssum = p_st.tile([64, 1], F32)
            ssq = p_st.tile([64, 1], F32)
            nc.vector.memset(ssum[:], 0.0)
            nc.vector.memset(ssq[:], 0.0)

            # ---------- phase A: conv1 (im2col matmul) + local stats ----------
            for j in range(34):
                im = p_im.tile([27, YX], F16i)
                for dz in range(3):
                    for dy in range(3):
                        r0 = (dz * 3 + dy) * 3
                        nc.sync.dma_start(
                            out=im[r0:r0 + 3, :],
                            in_=bass.AP(
                                tensor=vol,
                                offset=(j + dz) * PYX + dy * 66,
                                ap=[[1, 3], [66, 64], [1, 64]],
                            ),
                        )
                hs = p_h.tile([64, YX], F32)
                for ci in range(8):
                    ps = p_ps.tile([64, 512], F32)
                    nc.tensor.matmul(
                        out=ps[:], lhsT=w1_t[:],
                        rhs=im[:, ci * 512:(ci + 1) * 512],
                        start=True, stop=True,
                    )
                    nc.scalar.activation(
                        out=hs[:, ci * 512:(ci + 1) * 512], in_=ps[:],
                        func=AF.Copy,
                    )
                nc.sync.dma_start(out=h_raw[j, :, :], in_=hs[:])
                if 1 <= j <= 32:  # owned slices only
                    red = p_h.tile([64, 1], F32, tag="red")
                    nc.vector.tensor_reduce(
                        out=red[:], in_=hs[:], axis=mybir.AxisListType.X,
                        op=OP.add)
                    nc.vector.tensor_tensor(
                        out=ssum[:], in0=ssum[:], in1=red[:], op=OP.add)
                    for ci in range(8):
                        sq = p_h.tile([64, 512], F32, tag="sq")
                        sl = slice(ci * 512, (ci + 1) * 512)
                        nc.vector.tensor_tensor(
                            out=sq[:], in0=hs[:, sl], in1=hs[:, sl],
                            op=OP.mult)
                        nc.vector.tensor_reduce(
                            out=red[:], in_=sq[:], axis=mybir.AxisListType.X,
                            op=OP.add)
                        nc.vector.tensor_tensor(
                            out=ssq[:], in0=ssq[:], in1=red[:], op=OP.add)

            # ---------- phase B: stats allreduce + bn coefficients ----------
            stl = p_st.tile([64, 2], F32)
            nc.vector.tensor_copy(out=stl[:, 0:1], in_=ssum[:])
            nc.vector.tensor_copy(out=stl[:, 1:2], in_=ssq[:])
            nc.sync.dma_start(out=st_in[:, :], in_=stl[:])
            with tc.tile_critical():
                with nc.semaphore() as cc_sem:
                    nc.gpsimd.collective_compute(
                        "AllReduce", OP.add,
                        replica_groups=[list(range(N_CORES))],
                        ins=[st_in.ap().opt()], outs=[st_out.ap().opt()],
                    ).then_inc(cc_sem)
                    nc.gpsimd.wait_ge(cc_sem, 1)
            stg = p_st.tile([64, 2], F32)
            nc.sync.dma_start(out=stg[:], in_=st_out[:, :])
            mean = p_st.tile([64, 1], F32)
            nc.vector.tensor_scalar(
                out=mean[:], in0=stg[:, 0:1], scalar1=1.0 / NVOX_STATS,
                scalar2=None, op0=OP.mult)
            var = p_st.tile([64, 1], F32)
            nc.vector.tensor_scalar(
                out=var[:], in0=stg[:, 1:2], scalar1=1.0 / NVOX_STATS,
                scalar2=None, op0=OP.mult)
            m2 = p_st.tile([64, 1], F32)
            nc.vector.tensor_tensor(out=m2[:], in0=mean[:], in1=mean[:],
                                    op=OP.mult)
            nc.vector.tensor_tensor(out=var[:], in0=var[:], in1=m2[:],
                                    op=OP.subtract)
            nc.vector.tensor_scalar(
                out=var[:], in0=var[:], scalar1=float(EPS), scalar2=None,
                op0=OP.add)
            std = p_st.tile([64, 1], F32)
            nc.scalar.activation(out=std[:], in_=var[:], func=AF.Sqrt)
            rstd = p_st.tile([64, 1], F32)
            nc.vector.reciprocal(out=rstd[:], in_=std[:])
            scale = p_st.tile([64, 1], F32)
            nc.vector.tensor_tensor(out=scale[:], in0=gamma_t[:],
                                    in1=rstd[:], op=OP.mult)
            mb = p_st.tile([64, 1], F32)
            nc.vector.tensor_tensor(out=mb[:], in0=mean[:], in1=b1_t[:],
                                    op=OP.add)
            nc.vector.tensor_tensor(out=mb[:], in0=mb[:], in1=scale[:],
                                    op=OP.mult)
            shift = p_st.tile([64, 1], F32)
            nc.vector.tensor_tensor(out=shift[:], in0=beta_t[:], in1=mb[:],
                                    op=OP.subtract)

            # ---------- phase C: conv2 (27 PSUM-accumulated matmuls) ----------
            ring = p_ring.tile([64, 3 * PYX], F32)
            nc.vector.memset(ring[:], 0.0)
            ring_v = ring[:].rearrange("p (s y x) -> p s y x", s=3, y=66)

            def load_hp(j, slot):
                t = p_h.tile([64, YX], F32, tag="ld")
                nc.sync.dma_start(out=t[:], in_=h_raw[j, :, :])
                nc.vector.tensor_scalar(
                    out=t[:], in0=t[:], scalar1=scale[:], scalar2=shift[:],
                    op0=OP.mult, op1=OP.add)
                nc.scalar.activation(out=t[:], in_=t[:], func=AF.Relu)
                nc.vector.tensor_scalar(
                    out=ring_v[:, slot, 1:65, 1:65],
                    in0=t[:].rearrange("p (y x) -> p y x", y=64),
                    scalar1=hm_t[:, j:j + 1], scalar2=None, op0=OP.mult)

            load_hp(0, 0)
            load_hp(1, 1)
            load_hp(2, 2)
            for zo in range(32):
                if zo > 0:
                    load_hp(zo + 2, (zo + 2) % 3)
                oslice = p_o2.tile([3, YX], F16)
                occslice = p_o2.tile([1, YX], F16, tag="occs")
                for ci in range(8):
                    ps2 = p_ps.tile([3, 512], F32, tag="ps2")
                    for t in range(27):
                        dz, r = divmod(t, 9)
                        dy, dx = divmod(r, 3)
                        slot = (zo + dz) % 3
                        y0 = ci * 8 + dy
                        nc.tensor.matmul(
                            out=ps2[:],
                            lhsT=w2_t[:, t * 3:(t + 1) * 3],
                            rhs=ring_v[:, slot, y0:y0 + 8, dx:dx + 64],
                            start=(t == 0), stop=(t == 26),
                        )
                    nc.scalar.activation(
                        out=oslice[:, ci * 512:(ci + 1) * 512], in_=ps2[:],
                        func=AF.Identity, bias=b2_t[:])
                    nc.scalar.activation(
                        out=occslice[:, ci * 512:(ci + 1) * 512],
                        in_=ps2[0:1, :], func=AF.Sigmoid, bias=b2_t[0:1])
                if zo < 16:
                    nc.sync.dma_start(
                        out=out[:, zo * YX:(zo + 1) * YX], in_=oslice[:])
                else:
                    nc.sync.dma_start(
                        out=outh[:, (zo - 16) * YX:(zo - 15) * YX],
                        in_=oslice[:])
                nc.sync.dma_start(
                    out=occ[:, zo * YX:(zo + 1) * YX], in_=occslice[:])

    return nc


# ---------------------------------------------------------------------------
# walrus multi-wait workaround: split >1 sync-waits into EventSemaphores
# ---------------------------------------------------------------------------


def _install_bir_fix():
    import json
    import concourse.bass_utils as bu
    if getattr(bu, "_multiwait_patch", None):
        return

    def split_multiwaits(bir_json):
        bir = json.loads(bir_json)
        for fn in bir.get("functions", []):
            def walk(block):
                insts = block.get("instructions", [])
                outl = []
                for ins in insts:
                    waits = ins.get("sync_info", {}).get("on_wait", [])
                    if len(waits) > 1:
                        for i, w in enumerate(waits[1:]):
                            outl.append({
                                "debug": ins.get("debug", 0),
                                "engine": ins.get("engine"),
                                "ins": [], "outs": [],
                                "name": f"{ins.get('name', 'i')}_ws{i}",
                                "opcode": "EventSemaphore",
                                "sync_info": {"on_update": [],
                                              "on_wait": [w]},
                            })
                        ins["sync_info"]["on_wait"] = waits[:1]
                    outl.append(ins)
                block["instructions"] = outl
                for sub in block.get("blocks", []):
                    walk(sub)
            for b in fn.get("blocks", []):
                walk(b)
        return json.dumps(bir).encode()

    orig = bu.compile_bir_kernel

    def patched(bir_json, tmpdir, neff_name="file.neff", **kw):
        return orig(split_multiwaits(bir_json), tmpdir,
                    neff_name=neff_name, **kw)

    bu.compile_bir_kernel = patched
    bu._multiwait_patch = True
    import concourse.bass2jax as b2j
    b2j.compile_bir_kernel = patched


# ---------------------------------------------------------------------------
# cached PJRT dispatch
# ---------------------------------------------------------------------------


def _make_runner(nc, n_cores=N_CORES):
    import jax
    from jax.sharding import Mesh, PartitionSpec
    from jax.experimental.shard_map import shard_map
    import concourse.mybir as mybir
    from concourse.bass2jax import (
        _bass_exec_p, partition_id_tensor, install_neuronx_cc_hook,
    )

    install_neuronx_cc_hook()
    in_names, out_names, out_avals, zero_shapes = [], [], [], []
    for alloc in nc.m.functions[0].allocations:
        if not isinstance(alloc, mybir.MemoryLocationSet):
            continue
        name = alloc.memorylocations[0].name
        if alloc.kind == "ExternalInput":
            if (nc.partition_id_tensor is None
                    or name != nc.partition_id_tensor.name):
                in_names.append(name)
        elif alloc.kind == "ExternalOutput":
            shape = tuple(alloc.tensor_shape)
            out_names.append(name)
            out_avals.append(
                jax.core.ShapedArray(shape, mybir.dt.np(alloc.dtype)))
            zero_shapes.append((shape, mybir.dt.np(alloc.dtype)))
    n_params = len(in_names)
    all_in = in_names + out_names
    pname = nc.partition_id_tensor.name if nc.partition_id_tensor else None
    if pname:
        all_in = all_in + [pname]

    def _body(*args):
        operands = list(args)
        if pname:
            operands.append(partition_id_tensor())
        outs = _bass_exec_p.bind(
            *operands, out_avals=tuple(out_avals), in_names=tuple(all_in),
            out_names=tuple(out_names), lowering_input_output_aliases=(),
            sim_require_finite=False, sim_require_nnan=False, nc=nc)
        return tuple(outs)

    devices = jax.devices()[:n_cores]
    mesh = Mesh(np.asarray(devices), ("core",))
    nin = n_params + len(out_names)
    sharded = jax.jit(
        shard_map(_body, mesh=mesh,
                  in_specs=(PartitionSpec("core"),) * nin,
                  out_specs=(PartitionSpec("core"),) * len(out_names),
                  check_rep=False),
        keep_unused=True)

    from jax.sharding import NamedSharding
    zsh = NamedSharding(mesh, PartitionSpec("core"))
    zeros_dev = [
        jax.device_put(
            np.zeros((n_cores * s[0],) + tuple(s[1:]), dt), zsh)
        for s, dt in zero_shapes
    ]

    def run_async(in_maps):
        """Dispatch and return (jax outs, fetch) where fetch(name) -> np
        array [n_cores, ...] (only transfers the requested output)."""
        concat = [
            np.concatenate([np.asarray(m[name]) for m in in_maps], axis=0)
            for name in in_names
        ]
        outs = sharded(*concat, *zeros_dev)

        def fetch(name, core=None):
            i = out_names.index(name)
            s = zero_shapes[i][0]
            if core is None:
                return np.asarray(outs[i]).reshape((n_cores,) + s)
            return np.asarray(
                outs[i].addressable_shards[core].data).reshape(s)

        return outs, fetch

    return run_async


# ---------------------------------------------------------------------------
# host orchestration
# ---------------------------------------------------------------------------

_state = {}


def _get_runner():
    if "run" not in _state:
        import jax
        try:
            jax.config.update("jax_compilation_cache_dir", "/tmp/jaxcache")
            jax.config.update(
                "jax_persistent_cache_min_compile_time_secs", 0.0)
            jax.config.update(
                "jax_persistent_cache_min_entry_size_bytes", 0)
        except Exception:
            pass
        _install_bir_fix()
        nc = _build_nc()
        # Normalize debug paths/tracebacks so the serialized BIR (and hence
        # the jax persistent-cache key) is independent of where kernel.py
        # lives or who imported it.
        import re as _re
        _orig_tjb = nc.to_json_bytes

        def _sanitized_json_bytes():
            raw = _orig_tjb()
            raw = _re.sub(rb'"filename":\s*"[^"]*"', b'"filename": "k"', raw)
            raw = _re.sub(rb'"ant_traceback":\s*"(?:[^"\\]|\\.)*"',
                          b'"ant_traceback": ""', raw)
            return raw

        nc.to_json_bytes = _sanitized_json_bytes
        _state["run"] = _make_runner(nc)
    return _state["run"]


def _prep_w(w1, w2_full, b2_full):
    w1T = np.ascontiguousarray(
        np.asarray(w1, np.float32)[:, 0].reshape(64, 27).T).astype(np.float16)
    w2a = np.asarray(w2_full, np.float32)
    O = w2a.shape[0]
    wr = w2a.reshape(O, 64, 27)
    w2T = np.zeros((64, 81), np.float32)
    for t in range(27):
        for o in range(O):
            w2T[:, t * 3 + o] = wr[o, :, t]
    b2 = np.zeros((3, 1), np.float32)
    b2[:O, 0] = np.asarray(b2_full, np.float32)
    return w1T, w2T, b2


def _make_vol_inputs(volumes):
    vols, masks = [], []
    for c in range(N_CORES):
        b, s = c // 2, c % 2
        z0 = 32 * s
        Pfull = np.zeros((66, 66, 66), np.float32)
        Pfull[1:65, 1:65, 1:65] = volumes[b]
        slab = np.zeros((36, 66, 66), np.float32)
        lo = max(0, z0 - 1)
        hi = min(66, z0 + 35)
        slab[lo - (z0 - 1):hi - (z0 - 1)] = Pfull[lo:hi]
        vols.append(slab.reshape(36, PYX).astype(np.float16))
        hm = np.zeros((64, 34), np.float32)
        jj = np.arange(34)
        hm[:, (jj >= 1 - z0) & (jj <= 64 - z0)] = 1.0
        masks.append(hm)
    return vols, masks


def _dispatch_pass(run, volumes, w1T, w2T, b2, b1, gamma, beta):
    vols, masks = _make_vol_inputs(volumes)
    in_maps = [{
        "vol": vols[c], "w1": w1T,
        "b1": np.asarray(b1, np.float32).reshape(64, 1),
        "gamma": np.asarray(gamma, np.float32).reshape(64, 1),
        "beta": np.asarray(beta, np.float32).reshape(64, 1),
        "w2": w2T, "b2": b2, "hmask": masks[c],
    } for c in range(N_CORES)]
    return run(in_maps)


def _run_pass(run, volumes, w1T, w2T, b2, b1, gamma, beta):
    _, fetch = _dispatch_pass(run, volumes, w1T, w2T, b2, b1, gamma, beta)
    rlo, rhi = fetch("out"), fetch("outh")
    out = np.empty((4, 3, G, G, G), np.float32)
    for c in range(N_CORES):
        b, s = c // 2, c % 2
        out[b, :, 32 * s:32 * s + 16] = rlo[c].reshape(3, 16, G, G)
        out[b, :, 32 * s + 16:32 * s + 32] = rhi[c].reshape(3, 16, G, G)
    return out


def _kernel_device(points, ow1, ob1, ogamma, obeta, ow2, ob2,
                   dw1, db1, dgamma, dbeta, dw2, db2):
    from concurrent.futures import ThreadPoolExecutor
    points = np.asarray(points, np.float32)
    volumes = _np_voxelize(points)
    run = _get_runner()

    w1T, w2T, b2p = _prep_w(ow1, ow2, ob2)
    _, fetch = _dispatch_pass(run, volumes, w1T, w2T, b2p,
                              ob1, ogamma, obeta)

    lin = np.linspace(-1.0, 1.0, G, dtype=np.float32)
    zz, yy, xx = np.meshgrid(lin, lin, lin, indexing="ij")
    base = np.stack((zz, yy, xx), axis=-1)

    # overlap: fetch the hi half (background) while gathering the lo half.
    # half "lo" = each core's first 16 owned slices -> batch z in
    # [0,16) u [32,48); "hi" = [16,32) u [48,64).
    zlo = np.concatenate([np.arange(0, 16), np.arange(32, 48)])
    zhi = zlo + 16
    sampled = np.empty((4, 1, G, G, G), np.float32)

    def _gather_half(res, zidx):
        off = np.empty((4, 3, 32, G, G), np.float32)
        for c in range(N_CORES):
            b, s = c // 2, c % 2
            off[b, :, 16 * s:16 * s + 16] = res[c].reshape(3, 16, G, G)
        off_p = np.transpose(off, (0, 2, 3, 4, 1))
        grid = np.clip(base[None, zidx] + off_p * 0.1, -1.0, 1.0)
        sampled[:, :, zidx] = _np_grid_sample(volumes[:, None], grid)

    with ThreadPoolExecutor(max_workers=1) as ex:
        fut_lo = ex.submit(fetch, "out")
        fut_hi = ex.submit(fetch, "outh")
        _gather_half(fut_lo.result(), zlo)
        _gather_half(fut_hi.result(), zhi)

    w1T2, w2T2, b2p2 = _prep_w(dw1, dw2, db2)
    _, fetch = _dispatch_pass(run, sampled[:, 0], w1T2, w2T2, b2p2,
                              db1, dgamma, dbeta)
    res = fetch("occ")
    occ = np.empty((4, 1, G, G, G), np.float32)
    for c in range(N_CORES):
        b, s = c // 2, c % 2
        occ[b, 0, 32 * s:32 * s + 32] = res[c].reshape(32, G, G)
    return occ


def kernel(points, ow1, ob1, ogamma, obeta, ow2, ob2,
           dw1, db1, dgamma, dbeta, dw2, db2):
    if os.environ.get("P2M_FORCE_NUMPY", "0") != "1" and _state.get("ok", True):
        try:
            return _kernel_device(points, ow1, ob1, ogamma, obeta, ow2, ob2,
                                  dw1, db1, dgamma, dbeta, dw2, db2)
        except Exception:
            import traceback
            import sys as _sys
            traceback.print_exc()
            print("kernel: device path failed, numpy fallback",
                  file=_sys.stderr)
            _state["ok"] = False
    return _kernel_numpy(points, ow1, ob1, ogamma, obeta, ow2, ob2,
                         dw1, db1, dgamma, dbeta, dw2, db2)


# ---------------------------------------------------------------------------
# numpy fallback (baseline implementation)
# ---------------------------------------------------------------------------


def _np_conv3d(x, w, b):
    Bn, C, D, H, W = x.shape
    O = w.shape[0]
    V = D * H * W
    out = np.empty((Bn, O, D, H, W), np.float32)
    if C == 1:
        wm = w.reshape(O, 27)
        for bi in range(Bn):
            xp = np.pad(x[bi, 0], 1)
            col = np.empty((27, V), np.float32)
            t = 0
            for dz in range(3):
                for dy in range(3):
                    for dx in range(3):
                        col[t] = xp[dz:dz + D, dy:dy + H, dx:dx + W].ravel()
                        t += 1
            out[bi] = (wm @ col).reshape(O, D, H, W)
    else:
        wflat = np.ascontiguousarray(
            w.transpose(0, 2, 3, 4, 1).reshape(O * 27, C)).astype(np.float32)
        for bi in range(Bn):
            Y = (wflat @ x[bi].reshape(C, V)).reshape(O, 27, D, H, W)
            acc = np.zeros((O, D, H, W), np.float32)
            t = 0
            for dz in range(3):
                sz = dz - 1
                zo0, zo1 = max(0, -sz), D - max(0, sz)
                for dy in range(3):
                    sy = dy - 1
                    yo0, yo1 = max(0, -sy), H - max(0, sy)
                    for dx in range(3):
                        sx = dx - 1
                        xo0, xo1 = max(0, -sx), W - max(0, sx)
                        acc[:, zo0:zo1, yo0:yo1, xo0:xo1] += Y[
                            :, t, zo0 + sz:zo1 + sz, yo0 + sy:yo1 + sy,
                            xo0 + sx:xo1 + sx]
                        t += 1
            out[bi] = acc
    return out + b[None, :, None, None, None].astype(np.float32)


def _np_bn_relu(x, gamma, beta, eps=1e-5):
    Bn, C = x.shape[:2]
    xf = x.reshape(Bn, C, -1)
    cnt = Bn * xf.shape[2]
    s = np.einsum("bcv->c", xf, dtype=np.float64)
    ss = np.einsum("bcv,bcv->c", xf, xf, dtype=np.float64)
    m = s / cnt
    v = ss / cnt - m * m
    scale = (gamma.astype(np.float64) / np.sqrt(v + eps)).astype(np.float32)
    shift = (beta.astype(np.float64) - m * scale).astype(np.float32)
    out = x * scale[None, :, None, None, None]
    out += shift[None, :, None, None, None]
    return np.maximum(out, 0.0, out=out)


def _kernel_numpy(points, ow1, ob1, ogamma, obeta, ow2, ob2,
                  dw1, db1, dgamma, dbeta, dw2, db2):
    args = [np.asarray(a, np.float32) for a in
            (points, ow1, ob1, ogamma, obeta, ow2, ob2,
             dw1, db1, dgamma, dbeta, dw2, db2)]
    (points, ow1, ob1, ogamma, obeta, ow2, ob2,
     dw1, db1, dgamma, dbeta, dw2, db2) = args
    voxel = _np_voxelize(points)[:, None]
    h = _np_bn_relu(_np_conv3d(voxel, ow1, ob1), ogamma, obeta)
    offset = _np_conv3d(h, ow2, ob2)
    offset = np.transpose(offset, (0, 2, 3, 4, 1))
    lin = np.linspace(-1.0, 1.0, G, dtype=np.float32)
    zz, yy, xx = np.meshgrid(lin, lin, lin, indexing="ij")
    base = np.stack((zz, yy, xx), axis=-1)
    grid = np.clip(base[None] + offset * 0.1, -1.0, 1.0)
    sampled = _np_grid_sample(voxel, grid)
    h2 = _np_bn_relu(_np_conv3d(sampled, dw1, db1), dgamma, dbeta)
    z = _np_conv3d(h2, dw2, db2)
    return (1.0 / (1.0 + np.exp(-z))).astype(np.float32)


# ---------------------------------------------------------------------------
# import-time warmup: build + compile/load NEFF + one dummy dispatch, so the
# first kernel() call measures steady-state execution, not jit bring-up.
# ---------------------------------------------------------------------------

if os.environ.get("P2M_NO_WARMUP", "0") != "1":
    try:
        _run = _get_runner()
        _dummy_vols = np.zeros((4, G, G, G), np.float32)
        _w1T = np.zeros((27, 64), np.float32)
        _w2T = np.zeros((64, 81), np.float32)
        _b2 = np.zeros((3, 1), np.float32)
        _z64 = np.zeros(64, np.float32)
        _run_pass(_run, _dummy_vols, _w1T, _w2T, _b2, _z64,
                  np.ones(64, np.float32), _z64)
        _run_pass(_run, _dummy_vols, _w1T, _w2T, _b2, _z64,
                  np.ones(64, np.float32), _z64)
    except Exception:
        import traceback
        traceback.print_exc()
        _state["ok"] = False


# revision 16
# speedup vs baseline: 4.0930x; 4.0930x over previous
"""nn_PointCloud2Mesh kernel for 8 trn2 NeuronCores.

Pipeline: host voxelize (O(N) binning) -> device pass 1 (conv1 -> BN with
cross-core stats allreduce -> ReLU -> conv2 = offset field) -> host trilinear
grid_sample -> device pass 2 (same NEFF: decoder convs) -> host sigmoid.

Sharding: core c of 8 handles batch c//2, z-slab c%2 (32 slices + halo).
Both device passes run one shared Bass NEFF on cores 0-7 via PJRT; BN uses
an 8-core AllReduce of per-channel sums.  Heavy compute (the 22 GFLOP of
3^3 convs) runs on the NeuronCores; scatter/gather stay on host where they
are O(N) cheap.

A numpy fallback covers any device-path failure.
"""
import os
import numpy as np

G = 64
B, N = 4, 200000
YX = G * G
PYX = 66 * 66
N_CORES = 8
EPS = 1e-5

# ---------------------------------------------------------------------------
# host-side reference pieces (voxelize / grid_sample) - cheap O(N) parts
# ---------------------------------------------------------------------------


def _np_voxelize(points):
    pmin = points.min(axis=1, keepdims=True)
    pmax = points.max(axis=1, keepdims=True)
    npts = (points - pmin) / (pmax - pmin + 1e-6) * 2.0 - 1.0
    # npts >= -1 so (npts+1)*0.5*G >= 0: int cast == floor
    idx = np.clip(((npts + 1.0) * (0.5 * G)).astype(np.int32), 0, G - 1)
    lin = (idx[..., 0] * G + idx[..., 1]) * G + idx[..., 2]
    nb = points.shape[0]
    lin = lin + (np.arange(nb, dtype=np.int64)[:, None] * (G * G * G))
    hist = np.bincount(lin.ravel(), minlength=nb * G * G * G)
    return hist.astype(np.float32).reshape(nb, G, G, G)


def _np_grid_sample(vol, grid):
    Bv, C, D, H, W = vol.shape
    oshape = (Bv, C) + grid.shape[1:4]

    def unnorm(c, size):
        u = ((c + 1.0) * size - 1.0) * 0.5
        return np.clip(u, 0.0, size - 1.0)

    ix = unnorm(grid[..., 0], W)
    iy = unnorm(grid[..., 1], H)
    iz = unnorm(grid[..., 2], D)
    ix0, iy0, iz0 = np.floor(ix), np.floor(iy), np.floor(iz)
    fx, fy, fz = ix - ix0, iy - iy0, iz - iz0
    flat = vol.reshape(Bv, C, -1)
    zc = [np.clip(iz0.astype(np.int32), 0, D - 1) * (H * W),
          np.clip(iz0.astype(np.int32) + 1, 0, D - 1) * (H * W)]
    yc = [np.clip(iy0.astype(np.int32), 0, H - 1) * W,
          np.clip(iy0.astype(np.int32) + 1, 0, H - 1) * W]
    xc = [np.clip(ix0.astype(np.int32), 0, W - 1),
          np.clip(ix0.astype(np.int32) + 1, 0, W - 1)]
    wzs = [1.0 - fz, fz]
    wys = [1.0 - fy, fy]
    wxs = [1.0 - fx, fx]
    out = np.zeros(oshape, vol.dtype)
    for kz in range(2):
        for ky in range(2):
            zy = zc[kz] + yc[ky]
            wzy = wzs[kz] * wys[ky]
            for kx in range(2):
                lin = (zy + xc[kx]).reshape(Bv, -1)
                g = np.take_along_axis(flat, lin[:, None, :], axis=2)
                out += g.reshape(oshape) * (wzy * wxs[kx])[:, None]
    return out


# ---------------------------------------------------------------------------
# Bass kernel (built lazily; shared by encoder and decoder passes)
# ---------------------------------------------------------------------------


def _build_nc():
    import concourse.bass as bass
    import concourse.mybir as mybir
    from concourse.tile import TileContext

    F32 = mybir.dt.float32
    AF = mybir.ActivationFunctionType
    OP = mybir.AluOpType
    NVOX_STATS = float(4 * G * G * G)

    nc = bass.Bass("TRN2", target_bir_lowering=False,
                   disable_frame_to_traceback=True)

    # vol row r (r=0..35) = padded z index (z0-1+r) of the 66^3 zero-padded
    # volume (rows outside [0,66) zero).  h slice j (0..33) = conv1 output at
    # global z = z0-1+j, from vol rows j..j+2.
    F16i = mybir.dt.float16
    vol = nc.dram_tensor("vol", [36, PYX], F16i, kind="ExternalInput")
    w1 = nc.dram_tensor("w1", [27, 64], F16i, kind="ExternalInput")
    b1 = nc.dram_tensor("b1", [64, 1], F32, kind="ExternalInput")
    gamma = nc.dram_tensor("gamma", [64, 1], F32, kind="ExternalInput")
    beta = nc.dram_tensor("beta", [64, 1], F32, kind="ExternalInput")
    w2 = nc.dram_tensor("w2", [64, 81], F32, kind="ExternalInput")
    b2 = nc.dram_tensor("b2", [3, 1], F32, kind="ExternalInput")
    hmask = nc.dram_tensor("hmask", [64, 34], F32, kind="ExternalInput")
    BF16 = mybir.dt.bfloat16
    F16 = mybir.dt.float16
    out = nc.dram_tensor("out", [3, 16 * YX], F16, kind="ExternalOutput")
    outh = nc.dram_tensor("outh", [3, 16 * YX], F16, kind="ExternalOutput")
    # channel-0 occupancy = sigmoid(logit) written separately so the decoder
    # pass only downloads 1/3 of the bytes
    occ = nc.dram_tensor("occ", [1, 32 * YX], F16, kind="ExternalOutput")

    h_raw = nc.dram_tensor("h_raw", [34, 64, YX], F32)
    st_in = nc.dram_tensor("st_in", [64, 2], F32)
    st_out = nc.dram_tensor("st_out", [64, 2], F32)

    with TileContext(nc) as tc:
        with (
            tc.tile_pool(name="im2col", bufs=2) as p_im,
            tc.tile_pool(name="psum", bufs=4, space="PSUM") as p_ps,
            tc.tile_pool(name="hout", bufs=2) as p_h,
            tc.tile_pool(name="consts", bufs=1) as p_c,
            tc.tile_pool(name="stats", bufs=1) as p_st,
            tc.tile_pool(name="ring", bufs=1) as p_ring,
            tc.tile_pool(name="o2", bufs=2) as p_o2,
        ):
            w1_t = p_c.tile([27, 64], F16i)
            nc.sync.dma_start(out=w1_t[:], in_=w1[:, :])
            w2_t = p_c.tile([64, 81], F32)
            nc.sync.dma_start(out=w2_t[:], in_=w2[:, :])
            b1_t = p_c.tile([64, 1], F32)
            nc.sync.dma_start(out=b1_t[:], in_=b1[:, :])
            gamma_t = p_c.tile([64, 1], F32)
            nc.sync.dma_start(out=gamma_t[:], in_=gamma[:, :])
            beta_t = p_c.tile([64, 1], F32)
            nc.sync.dma_start(out=beta_t[:], in_=beta[:, :])
            b2_t = p_c.tile([3, 1], F32)
            nc.sync.dma_start(out=b2_t[:], in_=b2[:, :])
            hm_t = p_c.tile([64, 34], F32)
            nc.sync.dma_start(out=hm_t[:], in_=hmask[:, :])

            ssum = p_st.tile([64, 1], F32)
            ssq = p_st.tile([64, 1], F32)
            nc.vector.memset(ssum[:], 0.0)
            nc.vector.memset(ssq[:], 0.0)

            # ---------- phase A: conv1 (im2col matmul) + local stats ----------
            for j in range(34):
                im = p_im.tile([27, YX], F16i)
                for dz in range(3):
                    for dy in range(3):
                        r0 = (dz * 3 + dy) * 3
                        nc.sync.dma_start(
                            out=im[r0:r0 + 3, :],
                            in_=bass.AP(
                                tensor=vol,
                                offset=(j + dz) * PYX + dy * 66,
                                ap=[[1, 3], [66, 64], [1, 64]],
                            ),
                        )
                hs = p_h.tile([64, YX], F32)
                for ci in range(8):
                    ps = p_ps.tile([64, 512], F32)
                    nc.tensor.matmul(
                        out=ps[:], lhsT=w1_t[:],
                        rhs=im[:, ci * 512:(ci + 1) * 512],
                        start=True, stop=True,
                    )
                    nc.scalar.activation(
                        out=hs[:, ci * 512:(ci + 1) * 512], in_=ps[:],
                        func=AF.Copy,
                    )
                nc.sync.dma_start(out=h_raw[j, :, :], in_=hs[:])
                if 1 <= j <= 32:  # owned slices only
                    red = p_h.tile([64, 1], F32, tag="red")
                    nc.vector.tensor_reduce(
                        out=red[:], in_=hs[:], axis=mybir.AxisListType.X,
                        op=OP.add)
                    nc.vector.tensor_tensor(
                        out=ssum[:], in0=ssum[:], in1=red[:], op=OP.add)
                    for ci in range(8):
                        sq = p_h.tile([64, 512], F32, tag="sq")
                        sl = slice(ci * 512, (ci + 1) * 512)
                        nc.vector.tensor_tensor(
                            out=sq[:], in0=hs[:, sl], in1=hs[:, sl],
                            op=OP.mult)
                        nc.vector.tensor_reduce(
                            out=red[:], in_=sq[:], axis=mybir.AxisListType.X,
                            op=OP.add)
                        nc.vector.tensor_tensor(
                            out=ssq[:], in0=ssq[:], in1=red[:], op=OP.add)

            # ---------- phase B: stats allreduce + bn coefficients ----------
            stl = p_st.tile([64, 2], F32)
            nc.vector.tensor_copy(out=stl[:, 0:1], in_=ssum[:])
            nc.vector.tensor_copy(out=stl[:, 1:2], in_=ssq[:])
            nc.sync.dma_start(out=st_in[:, :], in_=stl[:])
            with tc.tile_critical():
                with nc.semaphore() as cc_sem:
                    nc.gpsimd.collective_compute(
                        "AllReduce", OP.add,
                        replica_groups=[list(range(N_CORES))],
                        ins=[st_in.ap().opt()], outs=[st_out.ap().opt()],
                    ).then_inc(cc_sem)
                    nc.gpsimd.wait_ge(cc_sem, 1)
            stg = p_st.tile([64, 2], F32)
            nc.sync.dma_start(out=stg[:], in_=st_out[:, :])
            mean = p_st.tile([64, 1], F32)
            nc.vector.tensor_scalar(
                out=mean[:], in0=stg[:, 0:1], scalar1=1.0 / NVOX_STATS,
                scalar2=None, op0=OP.mult)
            var = p_st.tile([64, 1], F32)
            nc.vector.tensor_scalar(
                out=var[:], in0=stg[:, 1:2], scalar1=1.0 / NVOX_STATS,
                scalar2=None, op0=OP.mult)
            m2 = p_st.tile([64, 1], F32)
            nc.vector.tensor_tensor(out=m2[:], in0=mean[:], in1=mean[:],
                                    op=OP.mult)
            nc.vector.tensor_tensor(out=var[:], in0=var[:], in1=m2[:],
                                    op=OP.subtract)
            nc.vector.tensor_scalar(
                out=var[:], in0=var[:], scalar1=float(EPS), scalar2=None,
                op0=OP.add)
            std = p_st.tile([64, 1], F32)
            nc.scalar.activation(out=std[:], in_=var[:], func=AF.Sqrt)
            rstd = p_st.tile([64, 1], F32)
            nc.vector.reciprocal(out=rstd[:], in_=std[:])
            scale = p_st.tile([64, 1], F32)
            nc.vector.tensor_tensor(out=scale[:], in0=gamma_t[:],
                                    in1=rstd[:], op=OP.mult)
            mb = p_st.tile([64, 1], F32)
            nc.vector.tensor_tensor(out=mb[:], in0=mean[:], in1=b1_t[:],
                                    op=OP.add)
            nc.vector.tensor_tensor(out=mb[:], in0=mb[:], in1=scale[:],
                                    op=OP.mult)
            shift = p_st.tile([64, 1], F32)
            nc.vector.tensor_tensor(out=shift[:], in0=beta_t[:], in1=mb[:],
                                    op=OP.subtract)

            # ---------- phase C: conv2 (27 PSUM-accumulated matmuls) ----------
            ring = p_ring.tile([64, 3 * PYX], F32)
            nc.vector.memset(ring[:], 0.0)
            ring_v = ring[:].rearrange("p (s y x) -> p s y x", s=3, y=66)

            def load_hp(j, slot):
                t = p_h.tile([64, YX], F32, tag="ld")
                nc.sync.dma_start(out=t[:], in_=h_raw[j, :, :])
                nc.vector.tensor_scalar(
                    out=t[:], in0=t[:], scalar1=scale[:], scalar2=shift[:],
                    op0=OP.mult, op1=OP.add)
                nc.scalar.activation(out=t[:], in_=t[:], func=AF.Relu)
                nc.vector.tensor_scalar(
                    out=ring_v[:, slot, 1:65, 1:65],
                    in0=t[:].rearrange("p (y x) -> p y x", y=64),
                    scalar1=hm_t[:, j:j + 1], scalar2=None, op0=OP.mult)

            load_hp(0, 0)
            load_hp(1, 1)
            load_hp(2, 2)
            for zo in range(32):
                if zo > 0:
                    load_hp(zo + 2, (zo + 2) % 3)
                oslice = p_o2.tile([3, YX], F16)
                occslice = p_o2.tile([1, YX], F16, tag="occs")
                for ci in range(8):
                    ps2 = p_ps.tile([3, 512], F32, tag="ps2")
                    for t in range(27):
                        dz, r = divmod(t, 9)
                        dy, dx = divmod(r, 3)
                        slot = (zo + dz) % 3
                        y0 = ci * 8 + dy
                        nc.tensor.matmul(
                            out=ps2[:],
                            lhsT=w2_t[:, t * 3:(t + 1) * 3],
                            rhs=ring_v[:, slot, y0:y0 + 8, dx:dx + 64],
                            start=(t == 0), stop=(t == 26),
                        )
                    nc.scalar.activation(
                        out=oslice[:, ci * 512:(ci + 1) * 512], in_=ps2[:],
                        func=AF.Identity, bias=b2_t[:])
                    nc.scalar.activation(
                        out=occslice[:, ci * 512:(ci + 1) * 512],
                        in_=ps2[0:1, :], func=AF.Sigmoid, bias=b2_t[0:1])
                if zo < 16:
                    nc.sync.dma_start(
                        out=out[:, zo * YX:(zo + 1) * YX], in_=oslice[:])
                else:
                    nc.sync.dma_start(
                        out=outh[:, (zo - 16) * YX:(zo - 15) * YX],
                        in_=oslice[:])
                nc.sync.dma_start(
                    out=occ[:, zo * YX:(zo + 1) * YX], in_=occslice[:])

    return nc


# ---------------------------------------------------------------------------
# walrus multi-wait workaround: split >1 sync-waits into EventSemaphores
# ---------------------------------------------------------------------------


def _install_bir_fix():
    import json
    import concourse.bass_utils as bu
    if getattr(bu, "_multiwait_patch", None):
        return

    def split_multiwaits(bir_json):
        bir = json.loads(bir_json)
        for fn in bir.get("functions", []):
            def walk(block):
                insts = block.get("instructions", [])
                outl = []
                for ins in insts:
                    waits = ins.get("sync_info", {}).get("on_wait", [])
                    if len(waits) > 1:
                        for i, w in enumerate(waits[1:]):
                            outl.append({
                                "debug": ins.get("debug", 0),
                                "engine": ins.get("engine"),
                                "ins": [], "outs": [],
                                "name": f"{ins.get('name', 'i')}_ws{i}",
                                "opcode": "EventSemaphore",
                                "sync_info": {"on_update": [],
                                              "on_wait": [w]},
                            })
                        ins["sync_info"]["on_wait"] = waits[:1]
                    outl.append(ins)
                block["instructions"] = outl
                for sub in block.get("blocks", []):
                    walk(sub)
            for b in fn.get("blocks", []):
                walk(b)
        return json.dumps(bir).encode()

    orig = bu.compile_bir_kernel

    def patched(bir_json, tmpdir, neff_name="file.neff", **kw):
        return orig(split_multiwaits(bir_json), tmpdir,
                    neff_name=neff_name, **kw)

    bu.compile_bir_kernel = patched
    bu._multiwait_patch = True
    import concourse.bass2jax as b2j
    b2j.compile_bir_kernel = patched


# ---------------------------------------------------------------------------
# cached PJRT dispatch
# ---------------------------------------------------------------------------


def _make_runner(nc, n_cores=N_CORES):
    import jax
    from jax.sharding import Mesh, PartitionSpec
    from jax.experimental.shard_map import shard_map
    import concourse.mybir as mybir
    from concourse.bass2jax import (
        _bass_exec_p, partition_id_tensor, install_neuronx_cc_hook,
    )

    install_neuronx_cc_hook()
    in_names, out_names, out_avals, zero_shapes = [], [], [], []
    for alloc in nc.m.functions[0].allocations:
        if not isinstance(alloc, mybir.MemoryLocationSet):
            continue
        name = alloc.memorylocations[0].name
        if alloc.kind == "ExternalInput":
            if (nc.partition_id_tensor is None
                    or name != nc.partition_id_tensor.name):
                in_names.append(name)
        elif alloc.kind == "ExternalOutput":
            shape = tuple(alloc.tensor_shape)
            out_names.append(name)
            out_avals.append(
                jax.core.ShapedArray(shape, mybir.dt.np(alloc.dtype)))
            zero_shapes.append((shape, mybir.dt.np(alloc.dtype)))
    n_params = len(in_names)
    all_in = in_names + out_names
    pname = nc.partition_id_tensor.name if nc.partition_id_tensor else None
    if pname:
        all_in = all_in + [pname]

    def _body(*args):
        operands = list(args)
        if pname:
            operands.append(partition_id_tensor())
        outs = _bass_exec_p.bind(
            *operands, out_avals=tuple(out_avals), in_names=tuple(all_in),
            out_names=tuple(out_names), lowering_input_output_aliases=(),
            sim_require_finite=False, sim_require_nnan=False, nc=nc)
        return tuple(outs)

    devices = jax.devices()[:n_cores]
    mesh = Mesh(np.asarray(devices), ("core",))
    nin = n_params + len(out_names)
    sharded = jax.jit(
        shard_map(_body, mesh=mesh,
                  in_specs=(PartitionSpec("core"),) * nin,
                  out_specs=(PartitionSpec("core"),) * len(out_names),
                  check_rep=False),
        keep_unused=True)

    from jax.sharding import NamedSharding
    zsh = NamedSharding(mesh, PartitionSpec("core"))
    zeros_dev = [
        jax.device_put(
            np.zeros((n_cores * s[0],) + tuple(s[1:]), dt), zsh)
        for s, dt in zero_shapes
    ]

    def run_async(in_maps):
        """Dispatch and return (jax outs, fetch) where fetch(name) -> np
        array [n_cores, ...] (only transfers the requested output)."""
        concat = [
            np.concatenate([np.asarray(m[name]) for m in in_maps], axis=0)
            for name in in_names
        ]
        outs = sharded(*concat, *zeros_dev)

        def fetch(name, core=None):
            i = out_names.index(name)
            s = zero_shapes[i][0]
            if core is None:
                return np.asarray(outs[i]).reshape((n_cores,) + s)
            return np.asarray(
                outs[i].addressable_shards[core].data).reshape(s)

        return outs, fetch

    return run_async


# ---------------------------------------------------------------------------
# host orchestration
# ---------------------------------------------------------------------------

_state = {}


def _get_runner():
    if "run" not in _state:
        import jax
        try:
            jax.config.update("jax_compilation_cache_dir", "/tmp/jaxcache")
            jax.config.update(
                "jax_persistent_cache_min_compile_time_secs", 0.0)
            jax.config.update(
                "jax_persistent_cache_min_entry_size_bytes", 0)
        except Exception:
            pass
        _install_bir_fix()
        nc = _build_nc()
        # Normalize debug paths/tracebacks so the serialized BIR (and hence
        # the jax persistent-cache key) is independent of where kernel.py
        # lives or who imported it.
        import re as _re
        _orig_tjb = nc.to_json_bytes

        def _sanitized_json_bytes():
            raw = _orig_tjb()
            raw = _re.sub(rb'"filename":\s*"[^"]*"', b'"filename": "k"', raw)
            raw = _re.sub(rb'"ant_traceback":\s*"(?:[^"\\]|\\.)*"',
                          b'"ant_traceback": ""', raw)
            return raw

        nc.to_json_bytes = _sanitized_json_bytes
        _state["run"] = _make_runner(nc)
    return _state["run"]


def _prep_w(w1, w2_full, b2_full):
    w1T = np.ascontiguousarray(
        np.asarray(w1, np.float32)[:, 0].reshape(64, 27).T).astype(np.float16)
    w2a = np.asarray(w2_full, np.float32)
    O = w2a.shape[0]
    wr = w2a.reshape(O, 64, 27)
    w2T = np.zeros((64, 81), np.float32)
    for t in range(27):
        for o in range(O):
            w2T[:, t * 3 + o] = wr[o, :, t]
    b2 = np.zeros((3, 1), np.float32)
    b2[:O, 0] = np.asarray(b2_full, np.float32)
    return w1T, w2T, b2


def _make_vol_inputs(volumes):
    vols, masks = [], []
    for c in range(N_CORES):
        b, s = c // 2, c % 2
        z0 = 32 * s
        Pfull = np.zeros((66, 66, 66), np.float32)
        Pfull[1:65, 1:65, 1:65] = volumes[b]
        slab = np.zeros((36, 66, 66), np.float32)
        lo = max(0, z0 - 1)
        hi = min(66, z0 + 35)
        slab[lo - (z0 - 1):hi - (z0 - 1)] = Pfull[lo:hi]
        vols.append(slab.reshape(36, PYX).astype(np.float16))
        hm = np.zeros((64, 34), np.float32)
        jj = np.arange(34)
        hm[:, (jj >= 1 - z0) & (jj <= 64 - z0)] = 1.0
        masks.append(hm)
    return vols, masks


def _dispatch_pass(run, volumes, w1T, w2T, b2, b1, gamma, beta):
    vols, masks = _make_vol_inputs(volumes)
    in_maps = [{
        "vol": vols[c], "w1": w1T,
        "b1": np.asarray(b1, np.float32).reshape(64, 1),
        "gamma": np.asarray(gamma, np.float32).reshape(64, 1),
        "beta": np.asarray(beta, np.float32).reshape(64, 1),
        "w2": w2T, "b2": b2, "hmask": masks[c],
    } for c in range(N_CORES)]
    return run(in_maps)


def _run_pass(run, volumes, w1T, w2T, b2, b1, gamma, beta):
    _, fetch = _dispatch_pass(run, volumes, w1T, w2T, b2, b1, gamma, beta)
    rlo, rhi = fetch("out"), fetch("outh")
    out = np.empty((4, 3, G, G, G), np.float32)
    for c in range(N_CORES):
        b, s = c // 2, c % 2
        out[b, :, 32 * s:32 * s + 16] = rlo[c].reshape(3, 16, G, G)
        out[b, :, 32 * s + 16:32 * s + 32] = rhi[c].reshape(3, 16, G, G)
    return out


def _kernel_device(points, ow1, ob1, ogamma, obeta, ow2, ob2,
                   dw1, db1, dgamma, dbeta, dw2, db2):
    from concurrent.futures import ThreadPoolExecutor
    points = np.asarray(points, np.float32)
    volumes = _np_voxelize(points)
    run = _get_runner()

    w1T, w2T, b2p = _prep_w(ow1, ow2, ob2)
    _, fetch = _dispatch_pass(run, volumes, w1T, w2T, b2p,
                              ob1, ogamma, obeta)

    lin = np.linspace(-1.0, 1.0, G, dtype=np.float32)
    zz, yy, xx = np.meshgrid(lin, lin, lin, indexing="ij")
    base = np.stack((zz, yy, xx), axis=-1)

    # overlap: fetch the hi half (background) while gathering the lo half.
    # half "lo" = each core's first 16 owned slices -> batch z in
    # [0,16) u [32,48); "hi" = [16,32) u [48,64).
    zlo = np.concatenate([np.arange(0, 16), np.arange(32, 48)])
    zhi = zlo + 16
    sampled = np.empty((4, 1, G, G, G), np.float32)

    def _gather_half(res, zidx):
        off = np.empty((4, 3, 32, G, G), np.float32)
        for c in range(N_CORES):
            b, s = c // 2, c % 2
            off[b, :, 16 * s:16 * s + 16] = res[c].reshape(3, 16, G, G)
        off_p = np.transpose(off, (0, 2, 3, 4, 1))
        grid = np.clip(base[None, zidx] + off_p * 0.1, -1.0, 1.0)
        sampled[:, :, zidx] = _np_grid_sample(volumes[:, None], grid)

    with ThreadPoolExecutor(max_workers=1) as ex:
        fut_lo = ex.submit(fetch, "out")
        fut_hi = ex.submit(fetch, "outh")
        _gather_half(fut_lo.result(), zlo)
        _gather_half(fut_hi.result(), zhi)

    w1T2, w2T2, b2p2 = _prep_w(dw1, dw2, db2)
    _, fetch = _dispatch_pass(run, sampled[:, 0], w1T2, w2T2, b2p2,
                              db1, dgamma, dbeta)
    res = fetch("occ")
    occ = np.empty((4, 1, G, G, G), np.float32)
    for c in range(N_CORES):
        b, s = c // 2, c % 2
        occ[b, 0, 32 * s:32 * s + 32] = res[c].reshape(32, G, G)
    return occ


def kernel(points, ow1, ob1, ogamma, obeta, ow2, ob2,
           dw1, db1, dgamma, dbeta, dw2, db2):
    if os.environ.get("P2M_FORCE_NUMPY", "0") != "1" and _state.get("ok", True):
        try:
            return _kernel_device(points, ow1, ob1, ogamma, obeta, ow2, ob2,
                                  dw1, db1, dgamma, dbeta, dw2, db2)
        except Exception:
            import traceback
            import sys as _sys
            traceback.print_exc()
            print("kernel: device path failed, numpy fallback",
                  file=_sys.stderr)
            _state["ok"] = False
    return _kernel_numpy(points, ow1, ob1, ogamma, obeta, ow2, ob2,
                         dw1, db1, dgamma, dbeta, dw2, db2)


# ---------------------------------------------------------------------------
# numpy fallback (baseline implementation)
# ---------------------------------------------------------------------------


def _np_conv3d(x, w, b):
    Bn, C, D, H, W = x.shape
    O = w.shape[0]
    V = D * H * W
    out = np.empty((Bn, O, D, H, W), np.float32)
    if C == 1:
        wm = w.reshape(O, 27)
        for bi in range(Bn):
            xp = np.pad(x[bi, 0], 1)
            col = np.empty((27, V), np.float32)
            t = 0
            for dz in range(3):
                for dy in range(3):
                    for dx in range(3):
                        col[t] = xp[dz:dz + D, dy:dy + H, dx:dx + W].ravel()
                        t += 1
            out[bi] = (wm @ col).reshape(O, D, H, W)
    else:
        wflat = np.ascontiguousarray(
            w.transpose(0, 2, 3, 4, 1).reshape(O * 27, C)).astype(np.float32)
        for bi in range(Bn):
            Y = (wflat @ x[bi].reshape(C, V)).reshape(O, 27, D, H, W)
            acc = np.zeros((O, D, H, W), np.float32)
            t = 0
            for dz in range(3):
                sz = dz - 1
                zo0, zo1 = max(0, -sz), D - max(0, sz)
                for dy in range(3):
                    sy = dy - 1
                    yo0, yo1 = max(0, -sy), H - max(0, sy)
                    for dx in range(3):
                        sx = dx - 1
                        xo0, xo1 = max(0, -sx), W - max(0, sx)
                        acc[:, zo0:zo1, yo0:yo1, xo0:xo1] += Y[
                            :, t, zo0 + sz:zo1 + sz, yo0 + sy:yo1 + sy,
                            xo0 + sx:xo1 + sx]
                        t += 1
            out[bi] = acc
    return out + b[None, :, None, None, None].astype(np.float32)


def _np_bn_relu(x, gamma, beta, eps=1e-5):
    Bn, C = x.shape[:2]
    xf = x.reshape(Bn, C, -1)
    cnt = Bn * xf.shape[2]
    s = np.einsum("bcv->c", xf, dtype=np.float64)
    ss = np.einsum("bcv,bcv->c", xf, xf, dtype=np.float64)
    m = s / cnt
    v = ss / cnt - m * m
    scale = (gamma.astype(np.float64) / np.sqrt(v + eps)).astype(np.float32)
    shift = (beta.astype(np.float64) - m * scale).astype(np.float32)
    out = x * scale[None, :, None, None, None]
    out += shift[None, :, None, None, None]
    return np.maximum(out, 0.0, out=out)


def _kernel_numpy(points, ow1, ob1, ogamma, obeta, ow2, ob2,
                  dw1, db1, dgamma, dbeta, dw2, db2):
    args = [np.asarray(a, np.float32) for a in
            (points, ow1, ob1, ogamma, obeta, ow2, ob2,
             dw1, db1, dgamma, dbeta, dw2, db2)]
    (points, ow1, ob1, ogamma, obeta, ow2, ob2,
     dw1, db1, dgamma, dbeta, dw2, db2) = args
    voxel = _np_voxelize(points)[:, None]
    h = _np_bn_relu(_np_conv3d(voxel, ow1, ob1), ogamma, obeta)
    offset = _np_conv3d(h, ow2, ob2)
    offset = np.transpose(offset, (0, 2, 3, 4, 1))
    lin = np.linspace(-1.0, 1.0, G, dtype=np.float32)
    zz, yy, xx = np.meshgrid(lin, lin, lin, indexing="ij")
    base = np.stack((zz, yy, xx), axis=-1)
    grid = np.clip(base[None] + offset * 0.1, -1.0, 1.0)
    sampled = _np_grid_sample(voxel, grid)
    h2 = _np_bn_relu(_np_conv3d(sampled, dw1, db1), dgamma, dbeta)
    z = _np_conv3d(h2, dw2, db2)
    return (1.0 / (1.0 + np.exp(-z))).astype(np.float32)


# ---------------------------------------------------------------------------
# import-time warmup: build + compile/load NEFF + one dummy dispatch, so the
# first kernel() call measures steady-state execution, not jit bring-up.
# ---------------------------------------------------------------------------

if os.environ.get("P2M_NO_WARMUP", "0") != "1":
    try:
        _run = _get_runner()
        _dummy_vols = np.zeros((4, G, G, G), np.float32)
        _w1T = np.zeros((27, 64), np.float32)
        _w2T = np.zeros((64, 81), np.float32)
        _b2 = np.zeros((3, 1), np.float32)
        _z64 = np.zeros(64, np.float32)
        _run_pass(_run, _dummy_vols, _w1T, _w2T, _b2, _z64,
                  np.ones(64, np.float32), _z64)
        _run_pass(_run, _dummy_vols, _w1T, _w2T, _b2, _z64,
                  np.ones(64, np.float32), _z64)
    except Exception:
        import traceback
        traceback.print_exc()
        _state["ok"] = False
